# revision 1
# baseline (speedup 1.0000x reference)
"""Trainium2 Bass kernel for grouped-query attention with qk-norm.

Problem (hardcoded): x(2,2048,1024) @ Wq(1024,1024) / Wkv(1024,512),
16 query heads, 4 kv heads, head_dim 64, k_scale(16,1,64) applied to the
group-broadcast k. Output (2,2048,1024).

Sharding: 8 cores = batch(2) x kv_heads(4). Each core computes its batch's
4 query heads against its kv head over the full 2048x2048 score matrix.

Device kernel layout choices:
- Host passes x transposed (xT: dim on partitions) so all projection
  matmuls contract over dim with no on-device transposes.
- k_scale is folded into Wk host-side: (x@Wk)*ks == x@(Wk*diag(ks)),
  giving a per-query-head scaled kkT directly from the projection.
- Scores are computed transposed (S^T: keys on partitions, queries free)
  so that exp(S^T) tiles feed the PV matmul directly as the moving
  operand (no P transpose).
- Softmax skips the max-subtraction (inputs are bounded; exp stays well
  inside fp32 range) and normalizes after PV via an appended ones-row in
  the V stationary operand (row 64 of the PV psum accumulates sum(exp)).
- Output is returned transposed per head (oT: 4*64 x 2048); the host
  transposes during the gather.
- Matmul inputs are float32r (fp32 storage, reduced-precision multiply,
  4x the fp32 throughput at moving-dim >= 256).
"""

import os
from contextlib import ExitStack

import numpy as np

import concourse.bacc as bacc
import concourse.mybir as mybir
import concourse.tile as tile
from concourse.bass_utils import run_bass_kernel_spmd

# Problem constants
B, N, DIM = 2, 2048, 1024
HEADS, KV_HEADS, DH = 16, 4, 64
G = HEADS // KV_HEADS  # query heads per kv head (4)
NCORES = 8
P = 128
KT = DIM // P  # 8 contraction tiles over dim
IC = 512  # query-chunk width
NI = N // IC  # 4
NJ = N // P  # 16 key tiles
SCALE = DH**-0.5

F32 = mybir.dt.float32
F32R = mybir.dt.float32r
F16 = mybir.dt.float16

# matmul input dtype: fp32r streams 1 row/cycle at N>=256 (4x faster than fp32)
USE_F32R = os.environ.get("KERNEL_F32", "0") != "1"
DMM = F32R if USE_F32R else F32


def emit_kernel(ctx, tc, xT, wq, wk, wv, eye, oT):
    nc = tc.nc
    Exp = mybir.ActivationFunctionType.Exp
    mult = mybir.AluOpType.mult

    def dr(ap):  # dram-side view matching the matmul dtype
        return ap.bitcast(DMM) if USE_F32R else ap

    wpool = ctx.enter_context(tc.tile_pool(name="w", bufs=1))
    qkpool = ctx.enter_context(tc.tile_pool(name="qk", bufs=1))
    ptpool = ctx.enter_context(tc.tile_pool(name="pt", bufs=6))
    npool = ctx.enter_context(tc.tile_pool(name="norm", bufs=2))

    # --- persistent SBUF tensors ---
    ones_sb = wpool.tile([P, DH], DMM, tag="ones")  # 1-row slices as bc lhsT
    eye_sb = wpool.tile([DH, DH], DMM, tag="eye")  # identity for vT transpose
    qT = [qkpool.tile([P, N], DMM, name=f"qT{hp}", tag=f"qT{hp}") for hp in range(2)]
    kkT = [qkpool.tile([P, N], DMM, name=f"kkT{hp}", tag=f"kkT{hp}") for hp in range(2)]
    vaug = qkpool.tile([P, NJ * (DH + 1)], F16, tag="vaug")
    nc.any.memset(vaug[:], 1.0)
    nc.any.memset(ones_sb[:].bitcast(F32), 1.0)
    warm = qkpool.tile([1, 1], F32, tag="warm")
    nc.scalar.activation(warm[:], ones_sb[0:1, 0:1].bitcast(F32), Exp)
    nc.sync.dma_start(eye_sb[:], dr(eye[:, :]))

    sums_d = nc.dram_tensor("sums_d", (G, N), F32, kind="ExternalOutput").ap()
    rec_d = nc.dram_tensor("rec_d", (G, N), F32, kind="ExternalOutput").ap()
    o_acc = [
        npool.tile([DH + 1, N], F32, name=f"oacc{h}", tag=f"oacc{h}", bufs=1)
        for h in range(G)
    ]
    rec_row = [
        npool.tile([1, N], F32, name=f"recrow{h}", tag=f"recrow{h}", bufs=1)
        for h in range(G)
    ]

    def recip_chunk(h, ic):
        # DVE reciprocal on a 1-row (1,2048) AP costs ~13us; on (128,4) it
        # is ~100x cheaper. The sums row is respread across partitions via
        # a DRAM bounce (DMA cannot repartition within SBUF).
        csl = slice(ic * IC, (ic + 1) * IC)
        sums_t = npool.tile([P, 4], F32, tag="sums_t", bufs=2)
        rec_t = npool.tile([P, 4], F32, tag="rec_t", bufs=2)
        nc.sync.dma_start(
            sums_t[:], sums_d[h : h + 1, csl].rearrange("o (p f) -> (o p) f", p=P)
        )
        nc.vector.reciprocal(rec_t[:], sums_t[:])
        nc.sync.dma_start(
            rec_d[h : h + 1, csl].rearrange("o (p f) -> (o p) f", p=P), rec_t[:]
        )
        nc.sync.dma_start(rec_row[h][0:1, csl], rec_d[h : h + 1, csl])

    def normalize_head(h, apsum):
        # GpSimd broadcasts the reciprocal row across partitions (PE-free).
        for ic in range(NI):
            csl = slice(ic * IC, (ic + 1) * IC)
            bc = npool.tile([DH, IC], F32, name="bcg", tag="bcg", bufs=4)
            nc.gpsimd.partition_broadcast(bc[:], rec_row[h][0:1, csl])
            fin = npool.tile([DH, IC], F32, tag="fin", bufs=4)
            nc.vector.tensor_tensor(fin[:], o_acc[h][0:DH, csl], bc[:], mult)
            nc.sync.dma_start(oT[h * DH : (h + 1) * DH, csl], fin[:])

    def qk_exp(hp, ic, jt, pt):
        csl = slice(ic * IC, (ic + 1) * IC)
        st = apsum.tile([P, 2 * IC], F32, tag="s", bufs=3, name="st")
        for half in range(2):
            rsl = slice(half * 64, half * 64 + 64)
            nc.tensor.matmul(
                st[:, half * IC : (half + 1) * IC],
                kkT[hp][rsl, jt * P : (jt + 1) * P],
                qT[hp][rsl, csl],
                start=True,
                stop=True,
                tile_position=(half * 64, 0),
            )
        nc.scalar.activation(pt[:], st[:], Exp, scale=SCALE)

    def pv_mm(o_ps, jt, pt):
        for half in range(2):
            nc.tensor.matmul(
                o_ps[half][:],
                vaug[:, jt * (DH + 1) : (jt + 1) * (DH + 1)],
                pt[:, half * IC : (half + 1) * IC],
                start=(jt == 0),
                stop=(jt == NJ - 1),
            )

    def attn_block(hp, ic, o_ps, jts):
        for jt in jts:
            pt = ptpool.tile([P, 2 * IC], F16, tag="pt")
            qk_exp(hp, ic, jt, pt)
            pv_mm(o_ps, jt, pt)

    def drain_block(hp, ic, o_ps):
        for half in range(2):
            h = 2 * hp + half
            nc.vector.tensor_copy(
                o_acc[h][:, ic * IC : (ic + 1) * IC], o_ps[half][:]
            )
            nc.sync.dma_start(
                sums_d[h : h + 1, ic * IC : (ic + 1) * IC],
                o_acc[h][DH : DH + 1, ic * IC : (ic + 1) * IC],
            )
            recip_chunk(h, ic)

    # S-tile pool lives for the whole kernel so early attention blocks can
    # overlap the projection phase (PV is deferred; its accumulator banks
    # open only after the projection psum pool closes).
    apsum = ctx.enter_context(tc.tile_pool(name="ap", bufs=3, space="PSUM"))
    # Dummy matmuls during the initial DMA wait keep the PE HAM activity
    # monitor busy so real projections start at 2.4GHz instead of 1.2.
    for _ in range(28):
        wt = apsum.tile([DH, IC], F32, tag="s", name="wt", bufs=3)
        nc.tensor.matmul(
            wt[:, 0:DH], ones_sb[:, 0:DH], ones_sb[:, 0:DH],
            start=True, stop=True,
        )

    # --- projections (fp16 inputs): qT / kkT (d on partitions) + vT ---
    with tc.tile_pool(name="xw", bufs=1) as xwpool:
        wq_sb = xwpool.tile([P, KT * 256], F16, tag="wq")
        wk_sb = xwpool.tile([P, KT * 256], F16, tag="wk")
        wv_sb = xwpool.tile([P, KT * DH], F16, tag="wv")
        xts = xwpool.tile([P, KT * N], F16, tag="xt")  # 4MB
        vT_sb = xwpool.tile([DH, N], DMM, tag="vT")

        def dma_x(kt, ic):
            r = slice(kt * P, (kt + 1) * P)
            csl = slice(ic * IC, (ic + 1) * IC)
            nc.gpsimd.dma_start(
                xts[:, kt * N + ic * IC : kt * N + (ic + 1) * IC], xT[r, csl]
            )

        for kt in range(KT):
            r = slice(kt * P, (kt + 1) * P)
            nc.sync.dma_start(wq_sb[:, kt * 256 : (kt + 1) * 256], wq[r, :])
            dma_x(kt, 0)
        for kt in range(KT):
            r = slice(kt * P, (kt + 1) * P)
            nc.sync.dma_start(wk_sb[:, kt * 256 : (kt + 1) * 256], wk[r, :])
            dma_x(kt, 1)
        for kt in range(KT):
            r = slice(kt * P, (kt + 1) * P)
            nc.sync.dma_start(wv_sb[:, kt * DH : (kt + 1) * DH], wv[r, :])
            dma_x(kt, 2)
        for kt in range(KT):
            dma_x(kt, 3)

        def proj_wave(ic, pp):
            # one wave = every projection chain that consumes xts chunk ic
            csl = slice(ic * IC, (ic + 1) * IC)
            for hp in range(2):
                for t, w_sb in ((qT[hp], wq_sb), (kkT[hp], wk_sb)):
                    ps = pp.tile([P, IC], F32, tag="pj", name="pjt", bufs=2)
                    for kt in range(KT):
                        c0 = kt * 256 + hp * 128
                        nc.tensor.matmul(
                            ps[:],
                            w_sb[:, c0 : c0 + 128],
                            xts[:, kt * N + ic * IC : kt * N + (ic + 1) * IC],
                            start=(kt == 0),
                            stop=(kt == KT - 1),
                        )
                    nc.vector.tensor_copy(t[:, csl], ps[:])
            ps = pp.tile([DH, IC], F32, tag="pj", name="pjv", bufs=2)
            for kt in range(KT):
                nc.tensor.matmul(
                    ps[:],
                    wv_sb[:, kt * DH : (kt + 1) * DH],
                    xts[:, kt * N + ic * IC : kt * N + (ic + 1) * IC],
                    start=(kt == 0),
                    stop=(kt == KT - 1),
                )
            nc.vector.tensor_copy(vT_sb[:, csl], ps[:])
            for jt in range(4 * ic, 4 * ic + 4):
                pv = pp.tile([P, DH], DMM, tag="pj", bufs=2, name="pvt")
                nc.tensor.transpose(
                    pv[:], vT_sb[:, jt * P : (jt + 1) * P], eye_sb[:]
                )
                nc.vector.tensor_copy(
                    vaug[:, jt * (DH + 1) : jt * (DH + 1) + DH], pv[:]
                )

        pt_hold = [
            ptpool.tile([P, 2 * IC], F16, name=f"pth{j}", tag=f"pth{j}", bufs=1)
            for j in range(8)
        ]
        with tc.tile_pool(name="pp", bufs=2, space="PSUM") as pp:
            proj_wave(0, pp)
            proj_wave(1, pp)
            # early QK+exp for (hp0, ic0) j-tiles 0-7 overlap the remaining
            # projection waves; their PV runs later (accumulator banks are
            # still occupied by the projection pool here).
            for jt in range(8):
                qk_exp(0, 0, jt, pt_hold[jt])
            proj_wave(2, pp)
            proj_wave(3, pp)

    # --- attention ---
    with tc.tile_pool(name="op", bufs=1, space="PSUM") as opool:
        for hp in range(2):
            for ic in range(NI):
                o_ps = [
                    opool.tile(
                        [DH + 1, IC], F32, name=f"ops{i}", tag=f"ops{i}", bufs=1
                    )
                    for i in range(2)
                ]
                if hp == 0 and ic == 0:
                    for jt in range(8):
                        pv_mm(o_ps, jt, pt_hold[jt])
                    attn_block(hp, ic, o_ps, range(8, NJ))
                else:
                    attn_block(hp, ic, o_ps, range(NJ))
                drain_block(hp, ic, o_ps)
        for h in range(G):
            normalize_head(h, opool)


_CACHE = {}


def build():
    if "nc" in _CACHE:
        return _CACHE["nc"]
    nc = bacc.Bacc(
        "TRN2", target_bir_lowering=False, debug=False, num_devices=NCORES
    )
    xT = nc.dram_tensor("xT", (DIM, N), F16, kind="ExternalInput").ap()
    wq = nc.dram_tensor("wq", (DIM, G * DH), F16, kind="ExternalInput").ap()
    wk = nc.dram_tensor("wk", (DIM, G * DH), F16, kind="ExternalInput").ap()
    wv = nc.dram_tensor("wv", (DIM, DH), F16, kind="ExternalInput").ap()
    eye = nc.dram_tensor("eye", (DH, DH), F32, kind="ExternalInput").ap()
    oT = nc.dram_tensor("oT", (G * DH, N), F32, kind="ExternalOutput").ap()
    with tile.TileContext(nc) as tc:
        with ExitStack() as ctx:
            emit_kernel(ctx, tc, xT, wq, wk, wv, eye, oT)
    nc.compile()
    _CACHE["nc"] = nc
    return nc


def make_in_maps(x, Wq, Wkv, k_scale):
    x = np.asarray(x, dtype=np.float32)
    Wq = np.asarray(Wq, dtype=np.float32)
    Wkv = np.asarray(Wkv, dtype=np.float32)
    k_scale = np.asarray(k_scale, dtype=np.float32)
    xTs = [np.ascontiguousarray(x[b].T) for b in range(B)]
    in_maps = []
    for c in range(NCORES):
        b, kv = divmod(c, KV_HEADS)
        wk_base = Wkv[:, kv * DH : (kv + 1) * DH]
        wk_c = np.concatenate(
            [wk_base * k_scale[kv * G + j, 0][None, :] for j in range(G)], axis=1
        )
        in_maps.append(
            {
                "xT": xTs[b].astype(np.float16),
                "wq": np.ascontiguousarray(Wq[:, kv * G * DH : (kv + 1) * G * DH]).astype(np.float16),
                "wk": np.ascontiguousarray(wk_c).astype(np.float16),
                "wv": np.ascontiguousarray(
                    Wkv[:, KV_HEADS * DH + kv * DH : KV_HEADS * DH + (kv + 1) * DH]
                ).astype(np.float16),
                "eye": np.eye(DH, dtype=np.float32),
            }
        )
    return in_maps


def gather(results):
    out = np.empty((B, N, HEADS * DH), dtype=np.float32)
    for c in range(NCORES):
        b, kv = divmod(c, KV_HEADS)
        out[b, :, kv * G * DH : (kv + 1) * G * DH] = results[c]["oT"].T
    return out


def kernel(x, Wq, Wkv, k_scale, _trace=False):
    nc = build()
    in_maps = make_in_maps(x, Wq, Wkv, k_scale)
    res = run_bass_kernel_spmd(
        nc, in_maps, core_ids=list(range(NCORES)), trace=_trace
    )
    out = gather(res.results)
    if _trace:
        kernel.last_result = res
    return out



# revision 2
# speedup vs baseline: 1.1341x; 1.1341x over previous
"""Trainium2 Bass kernel for grouped-query attention with qk-norm.

Problem (hardcoded): x(2,2048,1024) @ Wq(1024,1024) / Wkv(1024,512),
16 query heads, 4 kv heads, head_dim 64, k_scale(16,1,64) applied to the
group-broadcast k. Output (2,2048,1024).

Sharding: 8 cores = batch(2) x kv_heads(4). Each core computes its batch's
4 query heads against its kv head over the full 2048x2048 score matrix.

The kernel is scheduled around the Scalar (ACT) engine: softmax exp over
4 heads x 2048^2 scores is 16.8M activations ~= 143us of ACT busy time,
which is the critical path. Everything else (projections, QK/PV matmuls,
normalization) is interleaved so ACT never starves:
- Projection waves are emitted just-in-time inside the first attention
  block (kk chunk w gates QK of key tiles 4w..4w+3; v chunk w gates PV).
- Scores are computed transposed (S^T: keys on partitions) so exp(S^T)
  feeds PV directly as the moving operand.
- Softmax skips max-subtraction (inputs bounded) and normalizes after PV
  via an appended ones-row in the V stationary operand.
- Each 512-query block is drained, reciprocal'd and written out inline,
  overlapped under the next block's exp work.
- qT/kkT/v are fp16 (quantization ~5e-4 rel err, well under the 2e-2 gate).
- Optionally (KERNEL_NSCHRAUD>0) a subset of exp tiles is computed on the
  Vector engine via a Schraudolph-style exp2 bit trick (i16 = s*scale*
  log2(e)*1024 + 15*1024 - C, bitcast to fp16), relieving the ACT engine.
"""

import os
from contextlib import ExitStack

import numpy as np

import concourse.bacc as bacc
import concourse.mybir as mybir
import concourse.tile as tile
from concourse.bass_utils import run_bass_kernel_spmd

# Problem constants
B, N, DIM = 2, 2048, 1024
HEADS, KV_HEADS, DH = 16, 4, 64
G = HEADS // KV_HEADS  # query heads per kv head (4)
NCORES = 8
P = 128
KT = DIM // P  # 8 contraction tiles over dim
IC = 512  # query-chunk width
NI = N // IC  # 4
NJ = N // P  # 16 key tiles
NW = NJ // NI  # 4 key tiles per projection wave
SCALE = DH**-0.5

F32 = mybir.dt.float32
F16 = mybir.dt.float16
I16 = mybir.dt.int16

# Schraudolph exp2 offload: number of exp tiles per 16-j-tile block done on
# the Vector engine instead of ACT (0 disables).
NSCHRAUD = int(os.environ.get("KERNEL_NSCHRAUD", "0"))
SCH_C = float(os.environ.get("KERNEL_SCH_C", "45.0"))
SCH_MULT = SCALE * np.log2(np.e) * 1024.0
SCH_ADD = 15.0 * 1024.0 - SCH_C


def emit_kernel(ctx, tc, xT, wq, wk, wv, eye, oT):
    nc = tc.nc
    Exp = mybir.ActivationFunctionType.Exp
    mult = mybir.AluOpType.mult
    add = mybir.AluOpType.add

    wpool = ctx.enter_context(tc.tile_pool(name="w", bufs=1))
    qkpool = ctx.enter_context(tc.tile_pool(name="qk", bufs=1))
    ptpool = ctx.enter_context(tc.tile_pool(name="pt", bufs=6))
    npool = ctx.enter_context(tc.tile_pool(name="norm", bufs=2))
    # PSUM budget (8 banks of 2KB/partition):
    #   st scores  2 bufs x [128,1024] f32 = 4 banks
    #   o_ps       2 tags x [65,512]  f32 = 2 banks
    #   pj (proj)  2 bufs x [128,512] f32 = 2 banks
    apsum = ctx.enter_context(tc.tile_pool(name="ap", bufs=2, space="PSUM"))
    opool = ctx.enter_context(tc.tile_pool(name="op", bufs=1, space="PSUM"))
    ppool = ctx.enter_context(tc.tile_pool(name="pp", bufs=2, space="PSUM"))

    # --- persistent SBUF tensors ---
    ones_sb = wpool.tile([P, DH], F32, tag="ones")
    eye_sb = wpool.tile([DH, DH], F16, tag="eye")
    qT = [qkpool.tile([P, N], F16, name=f"qT{hp}", tag=f"qT{hp}") for hp in range(2)]
    kkT = [qkpool.tile([P, N], F16, name=f"kkT{hp}", tag=f"kkT{hp}") for hp in range(2)]
    vaug = qkpool.tile([P, NJ * (DH + 1)], F16, tag="vaug")
    wq_sb = qkpool.tile([P, KT * 256], F16, tag="wq")
    wk_sb = qkpool.tile([P, KT * 256], F16, tag="wk")
    wv_sb = qkpool.tile([P, KT * DH], F16, tag="wv")
    xts = qkpool.tile([P, KT * N], F16, tag="xt")  # 4MB
    vT_sb = qkpool.tile([DH, N], F16, tag="vT")

    nc.any.memset(vaug[:], 1.0)
    nc.any.memset(ones_sb[:], 1.0)
    warm = qkpool.tile([1, 1], F32, tag="warm")
    nc.scalar.activation(warm[:], ones_sb[0:1, 0:1], Exp)

    sums_d = nc.dram_tensor("sums_d", (G, N), F32, kind="ExternalOutput").ap()
    rec_d = nc.dram_tensor("rec_d", (G, N), F32, kind="ExternalOutput").ap()
    o_acc = [
        npool.tile([DH + 1, N], F32, name=f"oacc{h}", tag=f"oacc{h}", bufs=1)
        for h in range(G)
    ]
    rec_row = [
        npool.tile([1, N], F32, name=f"recrow{h}", tag=f"recrow{h}", bufs=1)
        for h in range(G)
    ]

    # --- DMA emission, priority order: first block's deps first ---
    nc.sync.dma_start(eye_sb[:], eye[:, :])

    def dma_w(w_sb, w_ap, kt, width):
        r = slice(kt * P, (kt + 1) * P)
        nc.sync.dma_start(w_sb[:, kt * width : (kt + 1) * width], w_ap[r, :])

    def dma_x(kt, ic):
        r = slice(kt * P, (kt + 1) * P)
        csl = slice(ic * IC, (ic + 1) * IC)
        nc.gpsimd.dma_start(
            xts[:, kt * N + ic * IC : kt * N + (ic + 1) * IC], xT[r, csl]
        )

    for kt in range(KT):
        dma_w(wk_sb, wk, kt, 256)
        dma_x(kt, 0)
    for kt in range(KT):
        dma_w(wq_sb, wq, kt, 256)
        dma_x(kt, 1)
    for kt in range(KT):
        dma_w(wv_sb, wv, kt, DH)
        dma_x(kt, 2)
    for kt in range(KT):
        dma_x(kt, 3)

    # Dummy matmuls during the initial DMA wait keep the PE HAM activity
    # monitor busy so real projections start at 2.4GHz instead of 1.2.
    for _ in range(24):
        wt = ppool.tile([DH, IC], F32, tag="pj", name="wt")
        nc.tensor.matmul(
            wt[:, 0:DH], ones_sb[:, 0:DH], ones_sb[:, 0:DH], start=True, stop=True
        )

    # --- projection waves (emitted JIT inside the attention loop) ---
    def proj_chain(dst, w_sb, c0, rows, ic):
        csl = slice(ic * IC, (ic + 1) * IC)
        ps = ppool.tile([rows, IC], F32, tag="pj", name="pjt")
        for kt in range(KT):
            nc.tensor.matmul(
                ps[:],
                w_sb[:, kt * 256 + c0 : kt * 256 + c0 + rows]
                if rows == P
                else w_sb[:, kt * DH : (kt + 1) * DH],
                xts[:, kt * N + ic * IC : kt * N + (ic + 1) * IC],
                start=(kt == 0),
                stop=(kt == KT - 1),
            )
        nc.vector.tensor_copy(dst[:, csl], ps[:])

    def kk_wave(ic):
        for hp in range(2):
            proj_chain(kkT[hp], wk_sb, hp * 128, P, ic)

    def q_wave(ic):
        for hp in range(2):
            proj_chain(qT[hp], wq_sb, hp * 128, P, ic)

    def v_wave(ic):
        proj_chain(vT_sb, wv_sb, 0, DH, ic)
        for jt in range(NW * ic, NW * ic + NW):
            pv = ppool.tile([P, DH], F16, tag="pj", name="pvt")
            nc.tensor.transpose(pv[:], vT_sb[:, jt * P : (jt + 1) * P], eye_sb[:])
            nc.vector.tensor_copy(
                vaug[:, jt * (DH + 1) : jt * (DH + 1) + DH], pv[:]
            )

    # --- attention primitives ---
    def qk_mm(hp, ic, jt):
        csl = slice(ic * IC, (ic + 1) * IC)
        st = apsum.tile([P, 2 * IC], F32, tag="s", bufs=2, name="st")
        for half in range(2):
            rsl = slice(half * 64, half * 64 + 64)
            nc.tensor.matmul(
                st[:, half * IC : (half + 1) * IC],
                kkT[hp][rsl, jt * P : (jt + 1) * P],
                qT[hp][rsl, csl],
                start=True,
                stop=True,
                tile_position=(half * 64, 0),
            )
        return st

    def exp_tile(st, use_dve):
        pt = ptpool.tile([P, 2 * IC], F16, tag="pt")
        if use_dve:
            nc.vector.tensor_scalar(
                pt[:].bitcast(I16), st[:], SCH_MULT, SCH_ADD, mult, add
            )
        else:
            nc.scalar.activation(pt[:], st[:], Exp, scale=SCALE)
        return pt

    def pv_mm(o_ps, jt, pt):
        for half in range(2):
            nc.tensor.matmul(
                o_ps[half][:],
                vaug[:, jt * (DH + 1) : (jt + 1) * (DH + 1)],
                pt[:, half * IC : (half + 1) * IC],
                start=(jt == 0),
                stop=(jt == NJ - 1),
            )

    def recip_chunk(h, ic):
        # DVE reciprocal on a 1-row (1,2048) AP costs ~13us; on (128,4) it
        # is ~100x cheaper. The sums row is respread across partitions via
        # a DRAM bounce (DMA cannot repartition within SBUF).
        csl = slice(ic * IC, (ic + 1) * IC)
        sums_t = npool.tile([P, 4], F32, tag="sums_t", bufs=2)
        rec_t = npool.tile([P, 4], F32, tag="rec_t", bufs=2)
        nc.sync.dma_start(
            sums_t[:], sums_d[h : h + 1, csl].rearrange("o (p f) -> (o p) f", p=P)
        )
        nc.vector.reciprocal(rec_t[:], sums_t[:])
        nc.sync.dma_start(
            rec_d[h : h + 1, csl].rearrange("o (p f) -> (o p) f", p=P), rec_t[:]
        )
        nc.sync.dma_start(rec_row[h][0:1, csl], rec_d[h : h + 1, csl])

    def normalize_chunk(h, ic):
        # GpSimd broadcasts the reciprocal row across partitions (PE-free).
        csl = slice(ic * IC, (ic + 1) * IC)
        bc = npool.tile([DH, IC], F32, name="bcg", tag="bcg", bufs=4)
        nc.gpsimd.partition_broadcast(bc[:], rec_row[h][0:1, csl])
        fin = npool.tile([DH, IC], F32, tag="fin", bufs=4)
        nc.vector.tensor_tensor(fin[:], o_acc[h][0:DH, csl], bc[:], mult)
        nc.sync.dma_start(oT[h * DH : (h + 1) * DH, csl], fin[:])

    def drain_block(hp, ic, o_ps):
        for half in range(2):
            h = 2 * hp + half
            nc.vector.tensor_copy(
                o_acc[h][:, ic * IC : (ic + 1) * IC], o_ps[half][:]
            )
            nc.sync.dma_start(
                sums_d[h : h + 1, ic * IC : (ic + 1) * IC],
                o_acc[h][DH : DH + 1, ic * IC : (ic + 1) * IC],
            )
            recip_chunk(h, ic)
            normalize_chunk(h, ic)

    # Schraudolph tile selection: spread DVE tiles across the back of each
    # block (they relieve ACT; placement is numerically irrelevant).
    sch_jts = set()
    if NSCHRAUD > 0:
        step = NJ // NSCHRAUD
        sch_jts = {NJ - 1 - i * step for i in range(NSCHRAUD)}

    # --- main loop: ACT-centric pipeline ---
    # Block (0,0) carries the projection waves JIT: kk chunk w must precede
    # QK of j-tiles 4w.., v chunk w must precede PV of j-tile 4w.
    kk_wave(0)
    q_wave(0)
    for hp in range(2):
        for ic in range(NI):
            o_ps = [
                opool.tile([DH + 1, IC], F32, name=f"ops{i}", tag=f"ops{i}", bufs=1)
                for i in range(2)
            ]
            if hp == 0 and ic > 0:
                q_wave(ic)
            pend = []  # (jt, pt) awaiting PV
            for jt in range(NJ):
                if hp == 0 and ic == 0:
                    if jt % NW == 0 and jt > 0:
                        kk_wave(jt // NW)
                st = qk_mm(hp, ic, jt)
                if hp == 0 and ic == 0 and jt % NW == 0:
                    v_wave(jt // NW)
                pend.append((jt, exp_tile(st, jt in sch_jts)))
                if len(pend) > 1:
                    j0, pt0 = pend.pop(0)
                    pv_mm(o_ps, j0, pt0)
            for j0, pt0 in pend:
                pv_mm(o_ps, j0, pt0)
            drain_block(hp, ic, o_ps)


_CACHE = {}


def build():
    if "nc" in _CACHE:
        return _CACHE["nc"]
    nc = bacc.Bacc(
        "TRN2", target_bir_lowering=False, debug=False, num_devices=NCORES
    )
    xT = nc.dram_tensor("xT", (DIM, N), F16, kind="ExternalInput").ap()
    wq = nc.dram_tensor("wq", (DIM, G * DH), F16, kind="ExternalInput").ap()
    wk = nc.dram_tensor("wk", (DIM, G * DH), F16, kind="ExternalInput").ap()
    wv = nc.dram_tensor("wv", (DIM, DH), F16, kind="ExternalInput").ap()
    eye = nc.dram_tensor("eye", (DH, DH), F16, kind="ExternalInput").ap()
    oT = nc.dram_tensor("oT", (G * DH, N), F32, kind="ExternalOutput").ap()
    with tile.TileContext(nc) as tc:
        with ExitStack() as ctx:
            emit_kernel(ctx, tc, xT, wq, wk, wv, eye, oT)
    nc.compile()
    _CACHE["nc"] = nc
    return nc


def make_in_maps(x, Wq, Wkv, k_scale):
    x = np.asarray(x, dtype=np.float32)
    Wq = np.asarray(Wq, dtype=np.float32)
    Wkv = np.asarray(Wkv, dtype=np.float32)
    k_scale = np.asarray(k_scale, dtype=np.float32)
    xTs = [np.ascontiguousarray(x[b].T) for b in range(B)]
    in_maps = []
    for c in range(NCORES):
        b, kv = divmod(c, KV_HEADS)
        wk_base = Wkv[:, kv * DH : (kv + 1) * DH]
        wk_c = np.concatenate(
            [wk_base * k_scale[kv * G + j, 0][None, :] for j in range(G)], axis=1
        )
        in_maps.append(
            {
                "xT": xTs[b].astype(np.float16),
                "wq": np.ascontiguousarray(
                    Wq[:, kv * G * DH : (kv + 1) * G * DH]
                ).astype(np.float16),
                "wk": np.ascontiguousarray(wk_c).astype(np.float16),
                "wv": np.ascontiguousarray(
                    Wkv[:, KV_HEADS * DH + kv * DH : KV_HEADS * DH + (kv + 1) * DH]
                ).astype(np.float16),
                "eye": np.eye(DH, dtype=np.float16),
            }
        )
    return in_maps


def gather(results):
    out = np.empty((B, N, HEADS * DH), dtype=np.float32)
    for c in range(NCORES):
        b, kv = divmod(c, KV_HEADS)
        out[b, :, kv * G * DH : (kv + 1) * G * DH] = results[c]["oT"].T
    return out


def kernel(x, Wq, Wkv, k_scale, _trace=False):
    nc = build()
    in_maps = make_in_maps(x, Wq, Wkv, k_scale)
    res = run_bass_kernel_spmd(
        nc, in_maps, core_ids=list(range(NCORES)), trace=_trace
    )
    out = gather(res.results)
    if _trace:
        kernel.last_result = res
    return out
